# revision 11
# baseline (speedup 1.0000x reference)
"""DecoderRNN (Bahdanau attention + LSTM cell + BatchNorm + vocab head) on 8 trn2 cores.

Strategy (no per-step feature traffic, 2 small all-gathers per step):
  - Precompute G[b, r, :] = features[b, r, :] @ W_id.T  (gate-dim sharded: each core
    holds its 256 gate channels) so the D=2048 contraction leaves the recurrent loop:
        ctx_t @ W_id.T = sum_r w_t[b, r] * G[b, r, :]
    computed per step as 49 diagonal-stationary matmuls accumulating in PSUM.
  - LSTM + BN1 run H-sharded (64 channels per core, full batch B=128 in the free dim);
    BN batch stats are per-channel so they stay local. Post-BN h is all-gathered
    (64x128 fp32 = 32KB per core).
  - Attention scoring is R-sharded (7 r-slots per core): tanh volume splits 8 ways;
    score slices are all-gathered (3.5KB per core) and softmax is replicated.
  - fc -> BN2 -> fc2 are off the critical path; fc2 is vocab-sharded (1250 per core).
  - All core-dependent behavior lives in DATA (per-core weight slices / fT slots
    49..55), the program is identical on all cores (SPMD).

Host does only linear re-layout: embedding lookup, transposes, slicing, mean_f/h0/c0,
bf16 cast of the feature stream.
"""

import os
import sys

sys.path.insert(0, "/opt/trn_rl_repo")

import numpy as np
import ml_dtypes

import concourse.bass as bass
import concourse.bacc as bacc
import concourse.mybir as mybir
import concourse.tile as tile
from contextlib import ExitStack
from concourse.bass_utils import run_bass_kernel_spmd

F32 = mybir.dt.float32
BF16 = mybir.dt.bfloat16
AF = mybir.ActivationFunctionType
OP = mybir.AluOpType

B, T, R, D, E, H, V = 128, 20, 49, 2048, 512, 512, 10000
EPS = 1e-5
NC = 8
RP = 56            # padded r slots (49 global + 7 local copies)
SLOTS = 7          # r's scored per core
HS = H // NC       # 64 h-channels per core
JS = 4 * HS        # 256 gate channels per core
VS = V // NC       # 1250 vocab per core
DCN = D // 128     # 16 contraction chunks over D
ECN = E // 128     # 4 chunks over E
HCN = H // 128     # 4 chunks over H

T_STEPS = int(os.environ.get("KERNEL_T_STEPS", T))

_cache = {}


def _bf(x):
    return np.ascontiguousarray(x.astype(ml_dtypes.bfloat16))


def _f32(x):
    return np.ascontiguousarray(x.astype(np.float32))


def build_nc(t_steps=T_STEPS):
    nc = bacc.Bacc("TRN2", target_bir_lowering=False, debug=False, num_devices=NC)

    def din(name, shape, dt=F32):
        return nc.dram_tensor(name, list(shape), dt, kind="ExternalInput")

    # ---- inputs (per-core data) ----
    fTd = din("fTd", [RP, 128, DCN * 128], BF16)      # fTd[s, dp, dc*128+b]
    WidT = din("WidT", [128, DCN, JS], BF16)
    WaT = din("WaT", [128, DCN, HCN, 128], BF16)
    weT = din("weT", [128, ECN, T, 128])
    UaT = din("UaT", [128, HCN, HCN, 128])
    WieT = din("WieT", [128, ECN, JS])
    WhhT = din("WhhT", [128, HCN, JS])
    gbias = din("gbias", [1, JS])
    vaT = din("vaT", [128, HCN])
    babu = din("babu", [128, HCN])
    fcwT = din("fcwT", [128, HCN, 2, 128])
    fcb = din("fcb", [1, 256])
    g1s = din("g1s", [HS, 1])
    be1s = din("be1s", [HS, 1])
    g2t = din("g2t", [128, 2])
    be2t = din("be2t", [128, 2])
    fc2T = din("fc2T", [128, 2, VS])
    fc2b = din("fc2b", [1, VS])
    h0T = din("h0T", [128, HCN, 128])
    c0s = din("c0s", [128, HS])
    eye_in = din("eye_in", [128, 128])

    # ---- outputs ----
    out_s = nc.dram_tensor("out_s", [t_steps, 128, VS], F32, kind="ExternalOutput")
    w_out = nc.dram_tensor("w_out", [t_steps, 128, R], F32, kind="ExternalOutput")
    debug = os.environ.get("KERNEL_DEBUG", "0") == "1"
    if debug:
        dbg_h0 = nc.dram_tensor("dbg_h0", [128, HCN * 128], F32, kind="ExternalOutput")
        dbg_att1 = nc.dram_tensor("dbg_att1", [128, HCN * SLOTS * 128], F32, kind="ExternalOutput")
        dbg_tb = nc.dram_tensor("dbg_tb", [128, HCN * SLOTS * 128], F32, kind="ExternalOutput")
        dbg_ssb = nc.dram_tensor("dbg_ssb", [RP, 128], F32, kind="ExternalOutput")
        dbg_gates = nc.dram_tensor("dbg_gates", [128, JS], F32, kind="ExternalOutput")
        dbg_hbn = nc.dram_tensor("dbg_hbn", [HS, 128], F32, kind="ExternalOutput")

    with tile.TileContext(nc) as tc, ExitStack() as ctx:
        cst = ctx.enter_context(tc.tile_pool(name="cst", bufs=1))
        dram = ctx.enter_context(tc.tile_pool(name="dram", bufs=2, space="DRAM"))

        def load(ap_dram, shape, dtype=F32, tag=None):
            t_ = cst.tile(list(shape), dtype, tag=tag)
            nc.sync.dma_start(t_[:], ap_dram)
            return t_

        # persistent constants in SBUF
        widt = load(WidT[:].rearrange("p a b -> p (a b)"), [128, DCN * JS], BF16, "widt")
        wat = load(WaT[:].rearrange("p a b c -> p (a b c)"), [128, DCN * HCN * 128], BF16, "wat")
        uat = load(UaT[:].rearrange("p a b c -> p (a b c)"), [128, HCN * HCN * 128], F32, "uat")
        wiet = load(WieT[:].rearrange("p a b -> p (a b)"), [128, ECN * JS], F32, "wiet")
        whht = load(WhhT[:].rearrange("p a b -> p (a b)"), [128, HCN * JS], F32, "whht")
        gbias_t = load(gbias[:], [1, JS], F32, "gbias")
        vat = load(vaT[:], [128, HCN], F32, "vat")
        babu_t = load(babu[:], [128, HCN], F32, "babu")
        fcwt = load(fcwT[:].rearrange("p a b c -> p (a b c)"), [128, HCN * 2 * 128], F32, "fcwt")
        fcb_t = load(fcb[:], [1, 256], F32, "fcb")
        g1_t = load(g1s[:], [HS, 1], F32, "g1")
        be1_t = load(be1s[:], [HS, 1], F32, "be1")
        g2_t = load(g2t[:], [128, 2], F32, "g2")
        be2_t = load(be2t[:], [128, 2], F32, "be2")
        fc2t = load(fc2T[:].rearrange("p a b -> p (a b)"), [128, 2 * VS], F32, "fc2t")
        fc2b_t = load(fc2b[:], [1, VS], F32, "fc2b")
        h0_t = load(h0T[:].rearrange("p a b -> p (a b)"), [128, HCN * 128], F32, "h0")
        c0_t = load(c0s[:], [128, HS], F32, "c0")
        eye = load(eye_in[:], [128, 128], F32, "eye")

        ones_row = cst.tile([1, 128], F32, tag="ones")
        nc.vector.memset(ones_row[:], 1.0)
        eps_t = cst.tile([128, 1], F32, tag="eps")
        nc.vector.memset(eps_t[:], EPS)

        G = cst.tile([128, R * JS], F32, tag="G")
        att1 = cst.tile([128, HCN * SLOTS * 128], F32, tag="att1")
        att1_v = att1[:].rearrange("p (h s b) -> p h s b", h=HCN, s=SLOTS, b=128)

        wid_v = widt[:].rearrange("p (a b) -> p a b", a=DCN, b=JS)
        wa_v = wat[:].rearrange("p (a b c) -> p a b c", a=DCN, b=HCN, c=128)
        ua_v = uat[:].rearrange("p (a b c) -> p a b c", a=HCN, b=HCN, c=128)
        wie_v = wiet[:].rearrange("p (a b) -> p a b", a=ECN, b=JS)
        whh_v = whht[:].rearrange("p (a b) -> p a b", a=HCN, b=JS)
        fcw_v = fcwt[:].rearrange("p (a b c) -> p a b c", a=HCN, b=2, c=128)
        fc2_v = fc2t[:].rearrange("p (a b) -> p a b", a=2, b=VS)
        h0_v = h0_t[:].rearrange("p (a b) -> p a b", a=HCN, b=128)
        G_v = G[:].rearrange("p (s j) -> p s j", s=R, j=JS)

        # ---- setup: stream fT once; G (slots 0..48) and att1 (slots 49..55) ----
        with tc.tile_pool(name="stage", bufs=3) as stg, \
             tc.tile_pool(name="psum_setup", bufs=2, space="PSUM") as psum_s:
            for s in range(RP):
                stage = stg.tile([128, DCN * 128], BF16, tag="stage")
                nc.sync.dma_start(stage[:], fTd[s])
                if s < R:
                    gps = psum_s.tile([128, JS], F32, tag="gps")
                    for dc in range(DCN):
                        nc.tensor.matmul(
                            gps[:], stage[:, dc * 128:(dc + 1) * 128], wid_v[:, dc],
                            start=(dc == 0), stop=(dc == DCN - 1))
                    if s % 2 == 0:
                        nc.vector.tensor_copy(G_v[:, s], gps[:])
                    else:
                        nc.scalar.copy(G_v[:, s], gps[:])
                else:
                    i = s - R
                    for hc in range(HCN):
                        aps = psum_s.tile([128, 128], F32, tag="aps")
                        for dc in range(DCN):
                            nc.tensor.matmul(
                                aps[:], wa_v[:, dc, hc], stage[:, dc * 128:(dc + 1) * 128],
                                start=(dc == 0), stop=(dc == DCN - 1))
                        # att1 = Wa@fT + (ba+bu), per-partition bias
                        nc.scalar.activation(att1_v[:, hc, i], aps[:], AF.Identity,
                                             bias=babu_t[:, hc:hc + 1])

        # ---- recurrent loop ----
        psum = ctx.enter_context(tc.tile_pool(name="psum", bufs=1, space="PSUM"))
        sb = ctx.enter_context(tc.tile_pool(name="sb", bufs=2))
        if debug:
            nc.sync.dma_start(dbg_h0[:], h0_t[:])
            nc.sync.dma_start(dbg_att1[:], att1[:])
        sb1 = ctx.enter_context(tc.tile_pool(name="sb1", bufs=1))
        dg_pool = ctx.enter_context(tc.tile_pool(name="dg", bufs=3))

        hT_prev = h0_v          # [128, hc, 128] view
        c_prev = c0_t[:]        # [128, 64]

        for t in range(t_steps):
            # --- a2 = Ua @ h (T-layout), then score slice = va . tanh(att1 + a2) ---
            tb = sb1.tile([128, HCN * SLOTS * 128], F32, tag="tanhbuf")
            tb_v = tb[:].rearrange("p (h s b) -> p h s b", h=HCN, s=SLOTS, b=128)
            sp0 = psum.tile([1, 512], F32, tag="sps0")
            sp1 = psum.tile([1, 384], F32, tag="sps1")
            for hc in range(HCN):
                a2p = psum.tile([128, 128], F32, tag="a2", bufs=2)
                for ec in range(HCN):
                    nc.tensor.matmul(a2p[:], ua_v[:, ec, hc], hT_prev[:, ec],
                                     start=(ec == 0), stop=(ec == HCN - 1))
                a2b = a2p[:].rearrange("p (o b) -> p o b", o=1, b=128).broadcast_to((128, SLOTS, 128))
                nc.vector.tensor_tensor(tb_v[:, hc], att1_v[:, hc], a2b, op=OP.add)
                nc.scalar.activation(tb_v[:, hc], tb_v[:, hc], AF.Tanh)
                tbf = tb_v[:, hc].rearrange("p s b -> p (s b)")
                nc.tensor.matmul(sp0[:], vat[:, hc:hc + 1], tbf[:, 0:512],
                                 start=(hc == 0), stop=(hc == HCN - 1))
                nc.tensor.matmul(sp1[:], vat[:, hc:hc + 1], tbf[:, 512:896],
                                 start=(hc == 0), stop=(hc == HCN - 1))
            # --- scores all-gather ---
            ssl = sb.tile([1, SLOTS * 128], F32, tag="ssl", bufs=1)
            nc.vector.tensor_copy(ssl[:, 0:512], sp0[:])
            nc.vector.tensor_copy(ssl[:, 512:896], sp1[:])
            sin = dram.tile([1, SLOTS * 128], F32, tag="sin")
            nc.sync.dma_start(sin[:], ssl[:])
            sout = dram.tile([RP, 128], F32, tag="sout")
            nc.gpsimd.collective_compute(
                "AllGather", OP.bypass, replica_groups=[list(range(NC))],
                ins=[sin[:].opt()], outs=[sout[:].opt()])
            ssb = sb.tile([RP, 128], F32, tag="ssb")
            nc.sync.dma_start(ssb[:], sout[:])
            if debug and t == 0:
                nc.sync.dma_start(dbg_tb[:], tb[:])
                nc.sync.dma_start(dbg_ssb[:], ssb[:])
            tps = psum.tile([128, RP], F32, tag="xpose")
            nc.tensor.transpose(tps[:], ssb[:], eye[0:RP, 0:RP])
            # --- softmax over the 49 real slots ---
            ex = sb.tile([128, R], F32, tag="ex")
            nc.scalar.activation(ex[:], tps[:, 0:R], AF.Exp)
            ssum = sb.tile([128, 1], F32, tag="ssum")
            nc.vector.tensor_reduce(ssum[:], ex[:], mybir.AxisListType.X, OP.add)
            rsum = sb.tile([128, 1], F32, tag="rsum")
            nc.vector.reciprocal(rsum[:], ssum[:])
            wgt = sb.tile([128, R], F32, tag="wgt")
            nc.vector.tensor_scalar_mul(wgt[:], ex[:], rsum[:])
            nc.sync.dma_start(w_out[t], wgt[:])

            # --- gates: bias + we@Wie + h@Whh + sum_r w_r * G_r ---
            wes = sb.tile([128, ECN * 128], F32, tag="wes")
            nc.sync.dma_start(
                wes[:].rearrange("p (a b) -> p a b", a=ECN, b=128), weT[:, :, t, :])
            wes_v = wes[:].rearrange("p (a b) -> p a b", a=ECN, b=128)
            gp = psum.tile([128, JS], F32, tag="gp", bufs=2)
            nc.tensor.matmul(gp[:], ones_row[:], gbias_t[:], start=True, stop=False)
            for ec in range(ECN):
                nc.tensor.matmul(gp[:], wes_v[:, ec], wie_v[:, ec], start=False, stop=False)
            for ec in range(HCN):
                nc.tensor.matmul(gp[:], hT_prev[:, ec], whh_v[:, ec], start=False, stop=False)
            for s in range(R):
                dgt = dg_pool.tile([128, 128], F32, tag="dg")
                if s % 3 == 2:
                    nc.scalar.mul(dgt[:], eye[:], wgt[:, s:s + 1])
                else:
                    nc.vector.tensor_scalar_mul(dgt[:], eye[:], wgt[:, s:s + 1])
                nc.tensor.matmul(gp[:], dgt[:], G_v[:, s], start=False, stop=(s == R - 1))

            if debug and t == 0:
                gdump = sb.tile([128, JS], F32, tag="gdump", bufs=1)
                nc.vector.tensor_copy(gdump[:], gp[:])
                nc.sync.dma_start(dbg_gates[:], gdump[:])
            # --- LSTM cell (local 64 channels), then BN1 ---
            ig = sb.tile([128, JS], F32, tag="ig")
            nc.scalar.activation(ig[:, 0:64], gp[:, 0:64], AF.Sigmoid)
            nc.scalar.activation(ig[:, 64:128], gp[:, 64:128], AF.Sigmoid)
            nc.scalar.activation(ig[:, 128:192], gp[:, 128:192], AF.Tanh)
            nc.scalar.activation(ig[:, 192:256], gp[:, 192:256], AF.Sigmoid)
            fc_ = sb.tile([128, HS], F32, tag="fc_")
            nc.vector.tensor_mul(fc_[:], ig[:, 64:128], c_prev)
            ig_ = sb.tile([128, HS], F32, tag="ig_")
            nc.vector.tensor_mul(ig_[:], ig[:, 0:64], ig[:, 128:192])
            c_new = sb.tile([128, HS], F32, tag="c")
            nc.vector.tensor_add(c_new[:], fc_[:], ig_[:])
            tc_ = sb.tile([128, HS], F32, tag="tc_")
            nc.scalar.activation(tc_[:], c_new[:], AF.Tanh)
            hraw = sb.tile([128, HS], F32, tag="hraw")
            nc.vector.tensor_mul(hraw[:], ig[:, 192:256], tc_[:])
            # transpose to [64, 128]
            hps = psum.tile([128, 128], F32, tag="xpose")
            nc.tensor.transpose(hps[0:HS, :], hraw[:], eye[:])
            hTr = sb.tile([HS, 128], F32, tag="hTr")
            nc.vector.tensor_copy(hTr[:], hps[0:HS, :])
            # BN1 (stats over batch = free dim)
            bns = sb.tile([HS, 6], F32, tag="bns")
            nc.vector.bn_stats(bns[:], hTr[:])
            bna = sb.tile([HS, 2], F32, tag="bna")
            nc.vector.bn_aggr(bna[:], bns[:])
            sq = sb.tile([HS, 1], F32, tag="sq")
            nc.scalar.activation(sq[:], bna[:, 1:2], AF.Sqrt, bias=eps_t[0:HS, :])
            rstd = sb.tile([HS, 1], F32, tag="rstd")
            nc.vector.reciprocal(rstd[:], sq[:])
            seff = sb.tile([HS, 1], F32, tag="seff")
            nc.vector.tensor_mul(seff[:], rstd[:], g1_t[:])
            mse = sb.tile([HS, 1], F32, tag="mse")
            nc.vector.tensor_mul(mse[:], bna[:, 0:1], seff[:])
            beff = sb.tile([HS, 1], F32, tag="beff")
            nc.vector.tensor_sub(beff[:], be1_t[:], mse[:])
            hbn = sb.tile([HS, 128], F32, tag="hbn")
            nc.scalar.activation(hbn[:], hTr[:], AF.Identity, bias=beff[:], scale=seff[:])
            # --- h all-gather ---
            hin = dram.tile([HS, 128], F32, tag="hin")
            nc.sync.dma_start(hin[:], hbn[:])
            if debug and t == 0:
                nc.sync.dma_start(dbg_hbn[:], hbn[:])
            hout = dram.tile([H, 128], F32, tag="hout")
            nc.gpsimd.collective_compute(
                "AllGather", OP.bypass, replica_groups=[list(range(NC))],
                ins=[hin[:].opt()], outs=[hout[:].opt()])
            hT_new = sb.tile([128, HCN * 128], F32, tag="hT")
            nc.sync.dma_start(
                hT_new[:].rearrange("p (a b) -> p a b", a=HCN, b=128),
                hout[:].rearrange("(a p) b -> p a b", a=HCN, p=128))
            hT_new_v = hT_new[:].rearrange("p (a b) -> p a b", a=HCN, b=128)

            # --- fc -> relu -> BN2 -> fc2 (off critical path) ---
            o1 = sb.tile([128, 2 * 128], F32, tag="o1")
            for mc in range(2):
                fps = psum.tile([128, 512], F32, tag="fv")
                nc.tensor.matmul(fps[:, 0:128], fcb_t[:, mc * 128:(mc + 1) * 128], ones_row[:],
                                 start=True, stop=False)
                for ec in range(HCN):
                    nc.tensor.matmul(fps[:, 0:128], fcw_v[:, ec, mc], hT_new_v[:, ec],
                                     start=False, stop=(ec == HCN - 1))
                nc.scalar.activation(o1[:, mc * 128:(mc + 1) * 128], fps[:, 0:128], AF.Relu)
            o1bn = sb.tile([128, 2 * 128], F32, tag="o1bn")
            for mc in range(2):
                o1c = o1[:, mc * 128:(mc + 1) * 128]
                bns2 = sb.tile([128, 6], F32, tag="bns2")
                nc.vector.bn_stats(bns2[:], o1c)
                bna2 = sb.tile([128, 2], F32, tag="bna2")
                nc.vector.bn_aggr(bna2[:], bns2[:])
                sq2 = sb.tile([128, 1], F32, tag="sq2")
                nc.scalar.activation(sq2[:], bna2[:, 1:2], AF.Sqrt, bias=eps_t[:])
                rstd2 = sb.tile([128, 1], F32, tag="rstd2")
                nc.vector.reciprocal(rstd2[:], sq2[:])
                seff2 = sb.tile([128, 1], F32, tag="seff2")
                nc.vector.tensor_mul(seff2[:], rstd2[:], g2_t[:, mc:mc + 1])
                mse2 = sb.tile([128, 1], F32, tag="mse2")
                nc.vector.tensor_mul(mse2[:], bna2[:, 0:1], seff2[:])
                beff2 = sb.tile([128, 1], F32, tag="beff2")
                nc.vector.tensor_sub(beff2[:], be2_t[:, mc:mc + 1], mse2[:])
                nc.scalar.activation(o1bn[:, mc * 128:(mc + 1) * 128], o1c, AF.Identity,
                                     bias=beff2[:], scale=seff2[:])
            for n0, nn in ((0, 512), (512, 512), (1024, VS - 1024)):
                vps = psum.tile([128, nn], F32, tag="fv")
                nc.tensor.matmul(vps[:], ones_row[:], fc2b_t[:, n0:n0 + nn],
                                 start=True, stop=False)
                for kc in range(2):
                    nc.tensor.matmul(vps[:], o1bn[:, kc * 128:(kc + 1) * 128],
                                     fc2_v[:, kc, n0:n0 + nn],
                                     start=False, stop=(kc == 1))
                vsb = sb.tile([128, 512], F32, tag="vsb")
                if n0 == 512:
                    nc.scalar.copy(vsb[:, 0:nn], vps[:])
                else:
                    nc.vector.tensor_copy(vsb[:, 0:nn], vps[:])
                nc.sync.dma_start(out_s[t][:, n0:n0 + nn], vsb[:, 0:nn])

            hT_prev = hT_new_v
            c_prev = c_new[:]

    nc.compile()
    return nc


def _host_prep(captions, features, emb, Wa, ba, Ua, bu, va, bv,
               W_ih, b_ih, W_hh, b_hh, g1, be1, fc_w, fc_b,
               g2, be2, fc2_w, fc2_b, ih_w, ih_b, ic_w, ic_b):
    f = _f32
    features = f(features)
    emb = f(emb)
    cap = np.asarray(captions).astype(np.int64)

    mean_f = features.mean(axis=1)                      # [B, D]
    h0 = mean_f @ f(ih_w).T + f(ih_b)                   # [B, H]
    c0 = mean_f @ f(ic_w).T + f(ic_b)
    embed = emb[cap]                                    # [B, T, E]

    W_ih = f(W_ih)
    W_ie = W_ih[:, :E]
    W_id = W_ih[:, E:]
    W_hh = f(W_hh)
    gb = f(b_ih) + f(b_hh)

    # fT staged: [slot, dp, dc*128 + b]; slots 0..48 = global r, 49..55 per-core
    fT = features.transpose(2, 1, 0)                    # [D, R, B]
    fT_sl = fT.reshape(DCN, 128, R, B).transpose(2, 1, 0, 3)   # [R, dp, dc, B]
    fT_sl = fT_sl.reshape(R, 128, DCN * 128)

    WaT_full = f(Wa).T                                   # [D, H]
    UaT_full = f(Ua).T                                   # [H, H]
    fcwT_full = f(fc_w).T                                # [H, 256]
    babu_full = f(ba) + f(bu)                            # [H]
    va_full = f(va)[0]                                   # [H]

    in_maps = []
    for k in range(NC):
        rows = np.concatenate([q * H + np.arange(HS) + k * HS for q in range(4)])
        my_r = [7 * k + i for i in range(SLOTS)]
        fT_core = np.zeros((RP, 128, DCN * 128), np.float32)
        fT_core[:R] = fT_sl
        for i, r in enumerate(my_r):
            if r < R:
                fT_core[R + i] = fT_sl[r]

        m = dict(
            fTd=_bf(fT_core),
            WidT=_bf(W_id[rows].T.reshape(DCN, 128, JS).transpose(1, 0, 2)),
            WaT=_bf(WaT_full.reshape(DCN, 128, HCN, 128).transpose(1, 0, 2, 3)),
            weT=f(embed.transpose(2, 1, 0).reshape(ECN, 128, T, B).transpose(1, 0, 2, 3)),
            UaT=f(UaT_full.reshape(HCN, 128, HCN, 128).transpose(1, 0, 2, 3)),
            WieT=f(W_ie[rows].T.reshape(ECN, 128, JS).transpose(1, 0, 2)),
            WhhT=f(W_hh[rows].T.reshape(HCN, 128, JS).transpose(1, 0, 2)),
            gbias=f(gb[rows][None, :]),
            vaT=f(va_full.reshape(HCN, 128).T),
            babu=f(babu_full.reshape(HCN, 128).T),
            fcwT=f(fcwT_full.reshape(HCN, 128, 2, 128).transpose(1, 0, 2, 3)),
            fcb=f(f(fc_b)[None, :]),
            g1s=f(f(g1)[k * HS:(k + 1) * HS][:, None]),
            be1s=f(f(be1)[k * HS:(k + 1) * HS][:, None]),
            g2t=f(f(g2).reshape(2, 128).T),
            be2t=f(f(be2).reshape(2, 128).T),
            fc2T=f(f(fc2_w)[k * VS:(k + 1) * VS].T.reshape(2, 128, VS).transpose(1, 0, 2)),
            fc2b=f(f(fc2_b)[k * VS:(k + 1) * VS][None, :]),
            h0T=f(h0.T.reshape(HCN, 128, B).transpose(1, 0, 2)),
            c0s=f(c0[:, k * HS:(k + 1) * HS]),
            eye_in=np.eye(128, dtype=np.float32),
        )
        in_maps.append(m)
    return in_maps


def kernel(**inputs):
    if "nc" not in _cache:
        _cache["nc"] = build_nc()
    nc = _cache["nc"]
    in_maps = _host_prep(**inputs)
    trace = os.environ.get("KERNEL_TRACE", "0") == "1"
    res = run_bass_kernel_spmd(nc, in_maps, core_ids=list(range(NC)), trace=trace)
    _cache["last_exec_ns"] = res.exec_time_ns
    outs = [res.results[k]["out_s"] for k in range(NC)]       # each [t, 128, VS]
    outputs = np.concatenate(outs, axis=2).transpose(1, 0, 2)  # [B, t, V]
    atten = res.results[0]["w_out"].transpose(1, 0, 2)         # [B, t, R]
    if outputs.shape[1] < T:
        pass  # debug mode with fewer steps
    return outputs.astype(np.float32), atten.astype(np.float32)


# revision 13
# speedup vs baseline: 1.4300x; 1.4300x over previous
"""DecoderRNN (Bahdanau attention + LSTM cell + BatchNorm + vocab head) on 8 trn2 cores.

Strategy (no per-step feature traffic, 2 small all-gathers per step):
  - Precompute G[b, r, :] = features[b, r, :] @ W_id.T  (gate-dim sharded: each core
    holds its 256 gate channels) so the D=2048 contraction leaves the recurrent loop:
        ctx_t @ W_id.T = sum_r w_t[b, r] * G[b, r, :]
    computed per step as 49 diagonal-stationary matmuls accumulating in PSUM.
  - LSTM + BN1 run H-sharded (64 channels per core, full batch B=128 in the free dim);
    BN batch stats are per-channel so they stay local. Post-BN h is all-gathered
    (64x128 fp32 = 32KB per core).
  - Attention scoring is R-sharded (7 r-slots per core): tanh volume splits 8 ways;
    score slices are all-gathered (3.5KB per core) and softmax is replicated.
  - fc -> BN2 -> fc2 are off the critical path; fc2 is vocab-sharded (1250 per core).
  - All core-dependent behavior lives in DATA (per-core weight slices / fT slots
    49..55), the program is identical on all cores (SPMD).

Host does only linear re-layout: embedding lookup, transposes, slicing, mean_f/h0/c0,
bf16 cast of the feature stream.
"""

import os
import sys

sys.path.insert(0, "/opt/trn_rl_repo")

import numpy as np
import ml_dtypes

import concourse.bass as bass
import concourse.bacc as bacc
import concourse.mybir as mybir
import concourse.tile as tile
from contextlib import ExitStack
from concourse.bass_utils import run_bass_kernel_spmd

F32 = mybir.dt.float32
BF16 = mybir.dt.bfloat16
AF = mybir.ActivationFunctionType
OP = mybir.AluOpType

B, T, R, D, E, H, V = 128, 20, 49, 2048, 512, 512, 10000
EPS = 1e-5
NC = 8
RP = 56            # padded r slots (49 global + 7 local copies)
SLOTS = 7          # r's scored per core
HS = H // NC       # 64 h-channels per core
JS = 4 * HS        # 256 gate channels per core
VS = V // NC       # 1250 vocab per core
DCN = D // 128     # 16 contraction chunks over D
ECN = E // 128     # 4 chunks over E
HCN = H // 128     # 4 chunks over H

T_STEPS = int(os.environ.get("KERNEL_T_STEPS", T))

_cache = {}


def _bf(x):
    return np.ascontiguousarray(x.astype(ml_dtypes.bfloat16))


def _f32(x):
    return np.ascontiguousarray(x.astype(np.float32))


def build_nc(t_steps=T_STEPS):
    nc = bacc.Bacc("TRN2", target_bir_lowering=False, debug=False, num_devices=NC)

    def din(name, shape, dt=F32):
        return nc.dram_tensor(name, list(shape), dt, kind="ExternalInput")

    # ---- inputs (per-core data) ----
    fTd = din("fTd", [RP, 128, DCN * 128], BF16)      # fTd[s, dp, dc*128+b]
    WidT = din("WidT", [128, DCN, JS], BF16)
    WaT = din("WaT", [128, DCN, HCN, 128], BF16)
    weT = din("weT", [128, ECN, T, 128], BF16)
    UaT = din("UaT", [128, HCN, HCN, 128], BF16)
    WieT = din("WieT", [128, ECN, JS], BF16)
    WhhT = din("WhhT", [128, HCN, JS], BF16)
    gbias = din("gbias", [1, JS], BF16)
    vaT = din("vaT", [128, HCN], BF16)
    babu = din("babu", [128, HCN])
    fcwT = din("fcwT", [128, HCN, 2, 128], BF16)
    fcb = din("fcb", [128, 2])
    g1s = din("g1s", [HS, 1])
    be1s = din("be1s", [HS, 1])
    g2t = din("g2t", [128, 2])
    be2t = din("be2t", [128, 2])
    fc2T = din("fc2T", [128, 2, VS], BF16)
    fc2b = din("fc2b", [128, VS], BF16)
    h0T = din("h0T", [128, HCN, 128], BF16)
    c0s = din("c0s", [128, HS])
    eye_in = din("eye_in", [128, 128])
    eyeb_in = din("eyeb_in", [128, 128], BF16)

    # ---- outputs ----
    out_s = nc.dram_tensor("out_s", [t_steps, 128, VS], F32, kind="ExternalOutput")
    w_out = nc.dram_tensor("w_out", [t_steps, 128, R], F32, kind="ExternalOutput")
    debug = os.environ.get("KERNEL_DEBUG", "0") == "1"
    if debug:
        dbg_h0 = nc.dram_tensor("dbg_h0", [128, HCN * 128], F32, kind="ExternalOutput")
        dbg_att1 = nc.dram_tensor("dbg_att1", [128, HCN * SLOTS * 128], F32, kind="ExternalOutput")
        dbg_tb = nc.dram_tensor("dbg_tb", [128, HCN * SLOTS * 128], F32, kind="ExternalOutput")
        dbg_ssb = nc.dram_tensor("dbg_ssb", [RP, 128], F32, kind="ExternalOutput")
        dbg_gates = nc.dram_tensor("dbg_gates", [128, JS], F32, kind="ExternalOutput")
        dbg_hbn = nc.dram_tensor("dbg_hbn", [HS, 128], F32, kind="ExternalOutput")

    with tile.TileContext(nc) as tc, ExitStack() as ctx:
        cst = ctx.enter_context(tc.tile_pool(name="cst", bufs=1))
        dram = ctx.enter_context(tc.tile_pool(name="dram", bufs=2, space="DRAM"))

        def load(ap_dram, shape, dtype=F32, tag=None):
            t_ = cst.tile(list(shape), dtype, tag=tag)
            nc.sync.dma_start(t_[:], ap_dram)
            return t_

        # persistent constants in SBUF
        widt = load(WidT[:].rearrange("p a b -> p (a b)"), [128, DCN * JS], BF16, "widt")
        wat = load(WaT[:].rearrange("p a b c -> p (a b c)"), [128, DCN * HCN * 128], BF16, "wat")
        uat = load(UaT[:].rearrange("p a b c -> p (a b c)"), [128, HCN * HCN * 128], BF16, "uat")
        wiet = load(WieT[:].rearrange("p a b -> p (a b)"), [128, ECN * JS], BF16, "wiet")
        whht = load(WhhT[:].rearrange("p a b -> p (a b)"), [128, HCN * JS], BF16, "whht")
        gbias_t = load(gbias[:], [1, JS], BF16, "gbias")
        vat = load(vaT[:], [128, HCN], BF16, "vat")
        babu_t = load(babu[:], [128, HCN], F32, "babu")
        fcwt = load(fcwT[:].rearrange("p a b c -> p (a b c)"), [128, HCN * 2 * 128], BF16, "fcwt")
        fcb_t = load(fcb[:], [128, 2], F32, "fcb")
        g1_t = load(g1s[:], [HS, 1], F32, "g1")
        be1_t = load(be1s[:], [HS, 1], F32, "be1")
        g2_t = load(g2t[:], [128, 2], F32, "g2")
        be2_t = load(be2t[:], [128, 2], F32, "be2")
        fc2t = load(fc2T[:].rearrange("p a b -> p (a b)"), [128, 2 * VS], BF16, "fc2t")
        fc2b_t = load(fc2b[:], [128, VS], BF16, "fc2b")
        h0_t = load(h0T[:].rearrange("p a b -> p (a b)"), [128, HCN * 128], BF16, "h0")
        c0_t = load(c0s[:], [128, HS], F32, "c0")
        eye = load(eye_in[:], [128, 128], F32, "eye")
        eyeb = load(eyeb_in[:], [128, 128], BF16, "eyeb")

        ones_row = cst.tile([1, 128], BF16, tag="ones")
        nc.vector.memset(ones_row[:], 1.0)
        eps_t = cst.tile([128, 1], F32, tag="eps")
        nc.vector.memset(eps_t[:], EPS)

        G = cst.tile([128, R * JS], BF16, tag="G")
        att1 = cst.tile([128, HCN * SLOTS * 128], F32, tag="att1")
        att1_v = att1[:].rearrange("p (h s b) -> p h s b", h=HCN, s=SLOTS, b=128)

        wid_v = widt[:].rearrange("p (a b) -> p a b", a=DCN, b=JS)
        wa_v = wat[:].rearrange("p (a b c) -> p a b c", a=DCN, b=HCN, c=128)
        ua_v = uat[:].rearrange("p (a b c) -> p a b c", a=HCN, b=HCN, c=128)
        wie_v = wiet[:].rearrange("p (a b) -> p a b", a=ECN, b=JS)
        whh_v = whht[:].rearrange("p (a b) -> p a b", a=HCN, b=JS)
        fcw_v = fcwt[:].rearrange("p (a b c) -> p a b c", a=HCN, b=2, c=128)
        fc2_v = fc2t[:].rearrange("p (a b) -> p a b", a=2, b=VS)
        h0_v = h0_t[:].rearrange("p (a b) -> p a b", a=HCN, b=128)
        G_v = G[:].rearrange("p (s j) -> p s j", s=R, j=JS)

        # ---- setup: stream fT once; G (slots 0..48) and att1 (slots 49..55) ----
        with tc.tile_pool(name="stage", bufs=3) as stg, \
             tc.tile_pool(name="psum_setup", bufs=2, space="PSUM") as psum_s:
            for s in range(RP):
                stage = stg.tile([128, DCN * 128], BF16, tag="stage")
                nc.sync.dma_start(stage[:], fTd[s])
                if s < R:
                    gps = psum_s.tile([128, JS], F32, tag="gps")
                    for dc in range(DCN):
                        nc.tensor.matmul(
                            gps[:], stage[:, dc * 128:(dc + 1) * 128], wid_v[:, dc],
                            start=(dc == 0), stop=(dc == DCN - 1))
                    if s % 2 == 0:
                        nc.vector.tensor_copy(G_v[:, s], gps[:])
                    else:
                        nc.scalar.copy(G_v[:, s], gps[:])
                else:
                    i = s - R
                    for hc in range(HCN):
                        aps = psum_s.tile([128, 128], F32, tag="aps")
                        for dc in range(DCN):
                            nc.tensor.matmul(
                                aps[:], wa_v[:, dc, hc], stage[:, dc * 128:(dc + 1) * 128],
                                start=(dc == 0), stop=(dc == DCN - 1))
                        # att1 = Wa@fT + (ba+bu), per-partition bias
                        nc.scalar.activation(att1_v[:, hc, i], aps[:], AF.Identity,
                                             bias=babu_t[:, hc:hc + 1])

        # ---- recurrent loop ----
        psum = ctx.enter_context(tc.tile_pool(name="psum", bufs=1, space="PSUM"))
        sb = ctx.enter_context(tc.tile_pool(name="sb", bufs=2))
        if debug:
            nc.sync.dma_start(dbg_h0[:], h0_t[:])
            nc.sync.dma_start(dbg_att1[:], att1[:])
        sb1 = ctx.enter_context(tc.tile_pool(name="sb1", bufs=1))
        dg_pool = ctx.enter_context(tc.tile_pool(name="dg", bufs=3))

        hT_prev = h0_v          # [128, hc, 128] view
        c_prev = c0_t[:]        # [128, 64]

        for t in range(t_steps):
            # --- a2 = Ua @ h (T-layout), then score slice = va . tanh(att1 + a2) ---
            tb = sb1.tile([128, HCN * SLOTS * 128], BF16, tag="tanhbuf")
            tb_v = tb[:].rearrange("p (h s b) -> p h s b", h=HCN, s=SLOTS, b=128)
            sp0 = psum.tile([1, 512], F32, tag="sps0")
            sp1 = psum.tile([1, 384], F32, tag="sps1")
            for hc in range(HCN):
                a2p = psum.tile([128, 128], F32, tag="a2", bufs=2)
                for ec in range(HCN):
                    nc.tensor.matmul(a2p[:], ua_v[:, ec, hc], hT_prev[:, ec],
                                     start=(ec == 0), stop=(ec == HCN - 1))
                a2b = a2p[:].rearrange("p (o b) -> p o b", o=1, b=128).broadcast_to((128, SLOTS, 128))
                nc.vector.tensor_tensor(tb_v[:, hc], att1_v[:, hc], a2b, op=OP.add)
                nc.scalar.activation(tb_v[:, hc], tb_v[:, hc], AF.Tanh)
                tbf = tb_v[:, hc].rearrange("p s b -> p (s b)")
                nc.tensor.matmul(sp0[:], vat[:, hc:hc + 1], tbf[:, 0:512],
                                 start=(hc == 0), stop=(hc == HCN - 1))
                nc.tensor.matmul(sp1[:], vat[:, hc:hc + 1], tbf[:, 512:896],
                                 start=(hc == 0), stop=(hc == HCN - 1))
            # --- scores all-gather ---
            ssl = sb.tile([1, SLOTS * 128], F32, tag="ssl", bufs=1)
            nc.vector.tensor_copy(ssl[:, 0:512], sp0[:])
            nc.vector.tensor_copy(ssl[:, 512:896], sp1[:])
            sin = dram.tile([1, SLOTS * 128], F32, tag="sin")
            nc.sync.dma_start(sin[:], ssl[:])
            sout = dram.tile([RP, 128], F32, tag="sout")
            nc.gpsimd.collective_compute(
                "AllGather", OP.bypass, replica_groups=[list(range(NC))],
                ins=[sin[:].opt()], outs=[sout[:].opt()])
            ssb = sb.tile([RP, 128], F32, tag="ssb")
            nc.sync.dma_start(ssb[:], sout[:])
            if debug and t == 0:
                nc.sync.dma_start(dbg_tb[:], tb[:])
                nc.sync.dma_start(dbg_ssb[:], ssb[:])
            tps = psum.tile([128, RP], F32, tag="xpose")
            nc.tensor.transpose(tps[:], ssb[:], eye[0:RP, 0:RP])
            # --- softmax over the 49 real slots ---
            ex = sb.tile([128, R], F32, tag="ex")
            nc.scalar.activation(ex[:], tps[:, 0:R], AF.Exp)
            ssum = sb.tile([128, 1], F32, tag="ssum")
            nc.vector.tensor_reduce(ssum[:], ex[:], mybir.AxisListType.X, OP.add)
            rsum = sb.tile([128, 1], F32, tag="rsum")
            nc.vector.reciprocal(rsum[:], ssum[:])
            wgt = sb.tile([128, R], F32, tag="wgt")
            nc.vector.tensor_scalar_mul(wgt[:], ex[:], rsum[:])
            nc.sync.dma_start(w_out[t], wgt[:])

            # --- gates: bias + we@Wie + h@Whh + sum_r w_r * G_r ---
            wes = sb.tile([128, ECN * 128], BF16, tag="wes")
            nc.sync.dma_start(
                wes[:].rearrange("p (a b) -> p a b", a=ECN, b=128), weT[:, :, t, :])
            wes_v = wes[:].rearrange("p (a b) -> p a b", a=ECN, b=128)
            gp = psum.tile([128, JS], F32, tag="gp", bufs=2)
            nc.tensor.matmul(gp[:], ones_row[:], gbias_t[:], start=True, stop=False)
            for ec in range(ECN):
                nc.tensor.matmul(gp[:], wes_v[:, ec], wie_v[:, ec], start=False, stop=False)
            for ec in range(HCN):
                nc.tensor.matmul(gp[:], hT_prev[:, ec], whh_v[:, ec], start=False, stop=False)
            for s in range(R):
                dgt = dg_pool.tile([128, 128], BF16, tag="dg")
                if s % 3 == 2:
                    nc.scalar.mul(dgt[:], eyeb[:], wgt[:, s:s + 1])
                else:
                    nc.vector.tensor_scalar_mul(dgt[:], eyeb[:], wgt[:, s:s + 1])
                nc.tensor.matmul(gp[:], dgt[:], G_v[:, s], start=False, stop=(s == R - 1))

            if debug and t == 0:
                gdump = sb.tile([128, JS], F32, tag="gdump", bufs=1)
                nc.vector.tensor_copy(gdump[:], gp[:])
                nc.sync.dma_start(dbg_gates[:], gdump[:])
            # --- LSTM cell (local 64 channels), then BN1 ---
            ig = sb.tile([128, JS], F32, tag="ig")
            nc.scalar.activation(ig[:, 0:64], gp[:, 0:64], AF.Sigmoid)
            nc.scalar.activation(ig[:, 64:128], gp[:, 64:128], AF.Sigmoid)
            nc.scalar.activation(ig[:, 128:192], gp[:, 128:192], AF.Tanh)
            nc.scalar.activation(ig[:, 192:256], gp[:, 192:256], AF.Sigmoid)
            fc_ = sb.tile([128, HS], F32, tag="fc_")
            nc.vector.tensor_mul(fc_[:], ig[:, 64:128], c_prev)
            ig_ = sb.tile([128, HS], F32, tag="ig_")
            nc.vector.tensor_mul(ig_[:], ig[:, 0:64], ig[:, 128:192])
            c_new = sb.tile([128, HS], F32, tag="c")
            nc.vector.tensor_add(c_new[:], fc_[:], ig_[:])
            tc_ = sb.tile([128, HS], F32, tag="tc_")
            nc.scalar.activation(tc_[:], c_new[:], AF.Tanh)
            hraw = sb.tile([128, HS], F32, tag="hraw")
            nc.vector.tensor_mul(hraw[:], ig[:, 192:256], tc_[:])
            # transpose to [64, 128]
            hps = psum.tile([128, 128], F32, tag="xpose")
            nc.tensor.transpose(hps[0:HS, :], hraw[:], eye[:])
            hTr = sb.tile([HS, 128], F32, tag="hTr")
            nc.vector.tensor_copy(hTr[:], hps[0:HS, :])
            # BN1 (stats over batch = free dim)
            bns = sb.tile([HS, 6], F32, tag="bns")
            nc.vector.bn_stats(bns[:], hTr[:])
            bna = sb.tile([HS, 2], F32, tag="bna")
            nc.vector.bn_aggr(bna[:], bns[:])
            sq = sb.tile([HS, 1], F32, tag="sq")
            nc.scalar.activation(sq[:], bna[:, 1:2], AF.Sqrt, bias=eps_t[0:HS, :])
            rstd = sb.tile([HS, 1], F32, tag="rstd")
            nc.vector.reciprocal(rstd[:], sq[:])
            seff = sb.tile([HS, 1], F32, tag="seff")
            nc.vector.tensor_mul(seff[:], rstd[:], g1_t[:])
            mse = sb.tile([HS, 1], F32, tag="mse")
            nc.vector.tensor_mul(mse[:], bna[:, 0:1], seff[:])
            beff = sb.tile([HS, 1], F32, tag="beff")
            nc.vector.tensor_sub(beff[:], be1_t[:], mse[:])
            hbn = sb.tile([HS, 128], BF16, tag="hbn")
            nc.scalar.activation(hbn[:], hTr[:], AF.Identity, bias=beff[:], scale=seff[:])
            # --- h all-gather ---
            hin = dram.tile([HS, 128], BF16, tag="hin")
            nc.sync.dma_start(hin[:], hbn[:])
            if debug and t == 0:
                nc.sync.dma_start(dbg_hbn[:], hbn[:])
            hout = dram.tile([H, 128], BF16, tag="hout")
            nc.gpsimd.collective_compute(
                "AllGather", OP.bypass, replica_groups=[list(range(NC))],
                ins=[hin[:].opt()], outs=[hout[:].opt()])
            hT_new = sb.tile([128, HCN * 128], BF16, tag="hT")
            nc.sync.dma_start(
                hT_new[:].rearrange("p (a b) -> p a b", a=HCN, b=128),
                hout[:].rearrange("(a p) b -> p a b", a=HCN, p=128))
            hT_new_v = hT_new[:].rearrange("p (a b) -> p a b", a=HCN, b=128)

            # --- fc -> relu -> BN2 -> fc2 (off critical path) ---
            o1 = sb.tile([128, 2 * 128], F32, tag="o1")
            for mc in range(2):
                fps = psum.tile([128, 512], F32, tag="fv")
                for ec in range(HCN):
                    nc.tensor.matmul(fps[:, 0:128], fcw_v[:, ec, mc], hT_new_v[:, ec],
                                     start=(ec == 0), stop=(ec == HCN - 1))
                nc.scalar.activation(o1[:, mc * 128:(mc + 1) * 128], fps[:, 0:128], AF.Relu,
                                     bias=fcb_t[:, mc:mc + 1])
            o1bn = sb.tile([128, 2 * 128], BF16, tag="o1bn")
            for mc in range(2):
                o1c = o1[:, mc * 128:(mc + 1) * 128]
                bns2 = sb.tile([128, 6], F32, tag="bns2")
                nc.vector.bn_stats(bns2[:], o1c)
                bna2 = sb.tile([128, 2], F32, tag="bna2")
                nc.vector.bn_aggr(bna2[:], bns2[:])
                sq2 = sb.tile([128, 1], F32, tag="sq2")
                nc.scalar.activation(sq2[:], bna2[:, 1:2], AF.Sqrt, bias=eps_t[:])
                rstd2 = sb.tile([128, 1], F32, tag="rstd2")
                nc.vector.reciprocal(rstd2[:], sq2[:])
                seff2 = sb.tile([128, 1], F32, tag="seff2")
                nc.vector.tensor_mul(seff2[:], rstd2[:], g2_t[:, mc:mc + 1])
                mse2 = sb.tile([128, 1], F32, tag="mse2")
                nc.vector.tensor_mul(mse2[:], bna2[:, 0:1], seff2[:])
                beff2 = sb.tile([128, 1], F32, tag="beff2")
                nc.vector.tensor_sub(beff2[:], be2_t[:, mc:mc + 1], mse2[:])
                nc.scalar.activation(o1bn[:, mc * 128:(mc + 1) * 128], o1c, AF.Identity,
                                     bias=beff2[:], scale=seff2[:])
            for n0, nn in ((0, 512), (512, 512), (1024, VS - 1024)):
                vps = psum.tile([128, nn], F32, tag="fv")
                for kc in range(2):
                    nc.tensor.matmul(vps[:], o1bn[:, kc * 128:(kc + 1) * 128],
                                     fc2_v[:, kc, n0:n0 + nn],
                                     start=(kc == 0), stop=(kc == 1))
                vsb = sb.tile([128, 512], F32, tag="vsb")
                nc.vector.tensor_tensor(vsb[:, 0:nn], vps[:], fc2b_t[:, n0:n0 + nn], op=OP.add)
                nc.sync.dma_start(out_s[t][:, n0:n0 + nn], vsb[:, 0:nn])

            hT_prev = hT_new_v
            c_prev = c_new[:]

    nc.compile()
    return nc


def _host_prep(captions, features, emb, Wa, ba, Ua, bu, va, bv,
               W_ih, b_ih, W_hh, b_hh, g1, be1, fc_w, fc_b,
               g2, be2, fc2_w, fc2_b, ih_w, ih_b, ic_w, ic_b):
    f = _f32
    features = f(features)
    emb = f(emb)
    cap = np.asarray(captions).astype(np.int64)

    mean_f = features.mean(axis=1)                      # [B, D]
    h0 = mean_f @ f(ih_w).T + f(ih_b)                   # [B, H]
    c0 = mean_f @ f(ic_w).T + f(ic_b)
    embed = emb[cap]                                    # [B, T, E]

    W_ih = f(W_ih)
    W_ie = W_ih[:, :E]
    W_id = W_ih[:, E:]
    W_hh = f(W_hh)
    gb = f(b_ih) + f(b_hh)

    # fT staged: [slot, dp, dc*128 + b]; slots 0..48 = global r, 49..55 per-core
    fT = features.transpose(2, 1, 0)                    # [D, R, B]
    fT_sl = fT.reshape(DCN, 128, R, B).transpose(2, 1, 0, 3)   # [R, dp, dc, B]
    fT_sl = fT_sl.reshape(R, 128, DCN * 128)

    WaT_full = f(Wa).T                                   # [D, H]
    UaT_full = f(Ua).T                                   # [H, H]
    fcwT_full = f(fc_w).T                                # [H, 256]
    babu_full = f(ba) + f(bu)                            # [H]
    va_full = f(va)[0]                                   # [H]

    in_maps = []
    for k in range(NC):
        rows = np.concatenate([q * H + np.arange(HS) + k * HS for q in range(4)])
        my_r = [7 * k + i for i in range(SLOTS)]
        fT_core = np.zeros((RP, 128, DCN * 128), np.float32)
        fT_core[:R] = fT_sl
        for i, r in enumerate(my_r):
            if r < R:
                fT_core[R + i] = fT_sl[r]

        m = dict(
            fTd=_bf(fT_core),
            WidT=_bf(W_id[rows].T.reshape(DCN, 128, JS).transpose(1, 0, 2)),
            WaT=_bf(WaT_full.reshape(DCN, 128, HCN, 128).transpose(1, 0, 2, 3)),
            weT=_bf(embed.transpose(2, 1, 0).reshape(ECN, 128, T, B).transpose(1, 0, 2, 3)),
            UaT=_bf(UaT_full.reshape(HCN, 128, HCN, 128).transpose(1, 0, 2, 3)),
            WieT=_bf(W_ie[rows].T.reshape(ECN, 128, JS).transpose(1, 0, 2)),
            WhhT=_bf(W_hh[rows].T.reshape(HCN, 128, JS).transpose(1, 0, 2)),
            gbias=_bf(gb[rows][None, :]),
            vaT=_bf(va_full.reshape(HCN, 128).T),
            babu=f(babu_full.reshape(HCN, 128).T),
            fcwT=_bf(fcwT_full.reshape(HCN, 128, 2, 128).transpose(1, 0, 2, 3)),
            fcb=f(f(fc_b).reshape(2, 128).T),
            g1s=f(f(g1)[k * HS:(k + 1) * HS][:, None]),
            be1s=f(f(be1)[k * HS:(k + 1) * HS][:, None]),
            g2t=f(f(g2).reshape(2, 128).T),
            be2t=f(f(be2).reshape(2, 128).T),
            fc2T=_bf(f(fc2_w)[k * VS:(k + 1) * VS].T.reshape(2, 128, VS).transpose(1, 0, 2)),
            fc2b=_bf(np.broadcast_to(f(fc2_b)[k * VS:(k + 1) * VS][None, :], (128, VS))),
            h0T=_bf(h0.T.reshape(HCN, 128, B).transpose(1, 0, 2)),
            c0s=f(c0[:, k * HS:(k + 1) * HS]),
            eye_in=np.eye(128, dtype=np.float32),
            eyeb_in=_bf(np.eye(128, dtype=np.float32)),
        )
        in_maps.append(m)
    return in_maps


def kernel(**inputs):
    if "nc" not in _cache:
        _cache["nc"] = build_nc()
    nc = _cache["nc"]
    in_maps = _host_prep(**inputs)
    trace = os.environ.get("KERNEL_TRACE", "0") == "1"
    res = run_bass_kernel_spmd(nc, in_maps, core_ids=list(range(NC)), trace=trace)
    _cache["last_exec_ns"] = res.exec_time_ns
    outs = [res.results[k]["out_s"] for k in range(NC)]       # each [t, 128, VS]
    outputs = np.concatenate(outs, axis=2).transpose(1, 0, 2)  # [B, t, V]
    atten = res.results[0]["w_out"].transpose(1, 0, 2)         # [B, t, R]
    if outputs.shape[1] < T:
        pass  # debug mode with fewer steps
    return outputs.astype(np.float32), atten.astype(np.float32)


# revision 15
# speedup vs baseline: 1.4407x; 1.0075x over previous
"""DecoderRNN (Bahdanau attention + LSTM cell + BatchNorm + vocab head) on 8 trn2 cores.

Strategy (no per-step feature traffic, 2 small all-gathers per step):
  - Precompute G[b, r, :] = features[b, r, :] @ W_id.T  (gate-dim sharded: each core
    holds its 256 gate channels) so the D=2048 contraction leaves the recurrent loop:
        ctx_t @ W_id.T = sum_r w_t[b, r] * G[b, r, :]
    computed per step as 49 diagonal-stationary matmuls accumulating in PSUM.
  - LSTM + BN1 run H-sharded (64 channels per core, full batch B=128 in the free dim);
    BN batch stats are per-channel so they stay local. Post-BN h is all-gathered
    (64x128 fp32 = 32KB per core).
  - Attention scoring is R-sharded (7 r-slots per core): tanh volume splits 8 ways;
    score slices are all-gathered (3.5KB per core) and softmax is replicated.
  - fc -> BN2 -> fc2 are off the critical path; fc2 is vocab-sharded (1250 per core).
  - All core-dependent behavior lives in DATA (per-core weight slices / fT slots
    49..55), the program is identical on all cores (SPMD).

Host does only linear re-layout: embedding lookup, transposes, slicing, mean_f/h0/c0,
bf16 cast of the feature stream.
"""

import os
import sys

sys.path.insert(0, "/opt/trn_rl_repo")

import numpy as np
import ml_dtypes

import concourse.bass as bass
import concourse.bacc as bacc
import concourse.mybir as mybir
import concourse.tile as tile
from contextlib import ExitStack
from concourse.bass_utils import run_bass_kernel_spmd

F32 = mybir.dt.float32
BF16 = mybir.dt.bfloat16
AF = mybir.ActivationFunctionType
OP = mybir.AluOpType

B, T, R, D, E, H, V = 128, 20, 49, 2048, 512, 512, 10000
EPS = 1e-5
NC = 8
RP = 56            # padded r slots (49 global + 7 local copies)
SLOTS = 7          # r's scored per core
HS = H // NC       # 64 h-channels per core
JS = 4 * HS        # 256 gate channels per core
VS = V // NC       # 1250 vocab per core
DCN = D // 128     # 16 contraction chunks over D
ECN = E // 128     # 4 chunks over E
HCN = H // 128     # 4 chunks over H

T_STEPS = int(os.environ.get("KERNEL_T_STEPS", T))

_cache = {}


def _bf(x):
    return np.ascontiguousarray(x.astype(ml_dtypes.bfloat16))


def _f32(x):
    return np.ascontiguousarray(x.astype(np.float32))


def build_nc(t_steps=T_STEPS):
    nc = bacc.Bacc("TRN2", target_bir_lowering=False, debug=False, num_devices=NC)

    def din(name, shape, dt=F32):
        return nc.dram_tensor(name, list(shape), dt, kind="ExternalInput")

    # ---- inputs (per-core data) ----
    fTd = din("fTd", [RP, 128, DCN * 128], BF16)      # fTd[s, dp, dc*128+b]
    WidT = din("WidT", [128, DCN, JS], BF16)
    WaT = din("WaT", [128, DCN, HCN, 128], BF16)
    weT = din("weT", [128, ECN, T, 128], BF16)
    UaT = din("UaT", [128, HCN, HCN, 128], BF16)
    WieT = din("WieT", [128, ECN, JS], BF16)
    WhhT = din("WhhT", [128, HCN, JS], BF16)
    gbias = din("gbias", [1, JS], BF16)
    vaT = din("vaT", [128, HCN], BF16)
    babu = din("babu", [128, HCN])
    fcwT = din("fcwT", [128, HCN, 2, 128], BF16)
    fcb = din("fcb", [128, 2])
    g1s = din("g1s", [HS, 1])
    be1s = din("be1s", [HS, 1])
    g2t = din("g2t", [128, 2])
    be2t = din("be2t", [128, 2])
    fc2T = din("fc2T", [128, 2, VS], BF16)
    fc2b = din("fc2b", [128, VS], BF16)
    h0T = din("h0T", [128, HCN, 128], BF16)
    c0s = din("c0s", [128, HS])
    eye_in = din("eye_in", [128, 128])
    eyeb_in = din("eyeb_in", [128, 128], BF16)

    # ---- outputs ----
    out_s = nc.dram_tensor("out_s", [t_steps, 128, VS], F32, kind="ExternalOutput")
    w_out = nc.dram_tensor("w_out", [t_steps, 128, R], F32, kind="ExternalOutput")
    debug = os.environ.get("KERNEL_DEBUG", "0") == "1"
    if debug:
        dbg_h0 = nc.dram_tensor("dbg_h0", [128, HCN * 128], F32, kind="ExternalOutput")
        dbg_att1 = nc.dram_tensor("dbg_att1", [128, HCN * SLOTS * 128], F32, kind="ExternalOutput")
        dbg_tb = nc.dram_tensor("dbg_tb", [128, HCN * SLOTS * 128], F32, kind="ExternalOutput")
        dbg_ssb = nc.dram_tensor("dbg_ssb", [RP, 128], F32, kind="ExternalOutput")
        dbg_gates = nc.dram_tensor("dbg_gates", [128, JS], F32, kind="ExternalOutput")
        dbg_hbn = nc.dram_tensor("dbg_hbn", [HS, 128], F32, kind="ExternalOutput")

    with tile.TileContext(nc) as tc, ExitStack() as ctx:
        cst = ctx.enter_context(tc.tile_pool(name="cst", bufs=1))
        dram = ctx.enter_context(tc.tile_pool(name="dram", bufs=2, space="DRAM"))

        def load(ap_dram, shape, dtype=F32, tag=None):
            t_ = cst.tile(list(shape), dtype, tag=tag)
            nc.sync.dma_start(t_[:], ap_dram)
            return t_

        # persistent constants in SBUF
        widt = load(WidT[:].rearrange("p a b -> p (a b)"), [128, DCN * JS], BF16, "widt")
        wat = load(WaT[:].rearrange("p a b c -> p (a b c)"), [128, DCN * HCN * 128], BF16, "wat")
        uat = load(UaT[:].rearrange("p a b c -> p (a b c)"), [128, HCN * HCN * 128], BF16, "uat")
        wiet = load(WieT[:].rearrange("p a b -> p (a b)"), [128, ECN * JS], BF16, "wiet")
        whht = load(WhhT[:].rearrange("p a b -> p (a b)"), [128, HCN * JS], BF16, "whht")
        gbias_t = load(gbias[:], [1, JS], BF16, "gbias")
        vat = load(vaT[:], [128, HCN], BF16, "vat")
        babu_t = load(babu[:], [128, HCN], F32, "babu")
        fcwt = load(fcwT[:].rearrange("p a b c -> p (a b c)"), [128, HCN * 2 * 128], BF16, "fcwt")
        fcb_t = load(fcb[:], [128, 2], F32, "fcb")
        g1_t = load(g1s[:], [HS, 1], F32, "g1")
        be1_t = load(be1s[:], [HS, 1], F32, "be1")
        g2_t = load(g2t[:], [128, 2], F32, "g2")
        be2_t = load(be2t[:], [128, 2], F32, "be2")
        fc2t = load(fc2T[:].rearrange("p a b -> p (a b)"), [128, 2 * VS], BF16, "fc2t")
        fc2b_t = load(fc2b[:], [128, VS], BF16, "fc2b")
        h0_t = load(h0T[:].rearrange("p a b -> p (a b)"), [128, HCN * 128], BF16, "h0")
        c0_t = load(c0s[:], [128, HS], F32, "c0")
        eye = load(eye_in[:], [128, 128], F32, "eye")
        eyeb = load(eyeb_in[:], [128, 128], BF16, "eyeb")

        ones_row = cst.tile([1, 128], BF16, tag="ones")
        nc.vector.memset(ones_row[:], 1.0)
        eps_t = cst.tile([128, 1], F32, tag="eps")
        nc.vector.memset(eps_t[:], EPS)

        G = cst.tile([128, R * JS], BF16, tag="G")
        att1 = cst.tile([128, HCN * SLOTS * 128], F32, tag="att1")
        att1_v = att1[:].rearrange("p (h s b) -> p h s b", h=HCN, s=SLOTS, b=128)

        wid_v = widt[:].rearrange("p (a b) -> p a b", a=DCN, b=JS)
        wa_v = wat[:].rearrange("p (a b c) -> p a b c", a=DCN, b=HCN, c=128)
        ua_v = uat[:].rearrange("p (a b c) -> p a b c", a=HCN, b=HCN, c=128)
        wie_v = wiet[:].rearrange("p (a b) -> p a b", a=ECN, b=JS)
        whh_v = whht[:].rearrange("p (a b) -> p a b", a=HCN, b=JS)
        fcw_v = fcwt[:].rearrange("p (a b c) -> p a b c", a=HCN, b=2, c=128)
        fc2_v = fc2t[:].rearrange("p (a b) -> p a b", a=2, b=VS)
        h0_v = h0_t[:].rearrange("p (a b) -> p a b", a=HCN, b=128)
        G_v = G[:].rearrange("p (s j) -> p s j", s=R, j=JS)

        # ---- setup: stream fT once; G (slots 0..48) and att1 (slots 49..55) ----
        with tc.tile_pool(name="stage", bufs=3) as stg, \
             tc.tile_pool(name="psum_setup", bufs=2, space="PSUM") as psum_s:
            for s in range(RP):
                stage = stg.tile([128, DCN * 128], BF16, tag="stage")
                nc.sync.dma_start(stage[:], fTd[s])
                if s < R:
                    gps = psum_s.tile([128, JS], F32, tag="gps")
                    for dc in range(DCN):
                        nc.tensor.matmul(
                            gps[:], stage[:, dc * 128:(dc + 1) * 128], wid_v[:, dc],
                            start=(dc == 0), stop=(dc == DCN - 1))
                    if s % 2 == 0:
                        nc.vector.tensor_copy(G_v[:, s], gps[:])
                    else:
                        nc.scalar.copy(G_v[:, s], gps[:])
                else:
                    i = s - R
                    for hc in range(HCN):
                        aps = psum_s.tile([128, 128], F32, tag="aps")
                        for dc in range(DCN):
                            nc.tensor.matmul(
                                aps[:], wa_v[:, dc, hc], stage[:, dc * 128:(dc + 1) * 128],
                                start=(dc == 0), stop=(dc == DCN - 1))
                        # att1 = Wa@fT + (ba+bu), per-partition bias
                        nc.scalar.activation(att1_v[:, hc, i], aps[:], AF.Identity,
                                             bias=babu_t[:, hc:hc + 1])

        # ---- recurrent loop ----
        psum = ctx.enter_context(tc.tile_pool(name="psum", bufs=1, space="PSUM"))
        sb = ctx.enter_context(tc.tile_pool(name="sb", bufs=2))
        if debug:
            nc.sync.dma_start(dbg_h0[:], h0_t[:])
            nc.sync.dma_start(dbg_att1[:], att1[:])
        sb1 = ctx.enter_context(tc.tile_pool(name="sb1", bufs=1))
        dg_pool = ctx.enter_context(tc.tile_pool(name="dg", bufs=3))

        hT_prev = h0_v          # [128, hc, 128] view
        c_prev = c0_t[:]        # [128, 64]

        for t in range(t_steps):
            # --- a2 = Ua @ h (T-layout), then score slice = va . tanh(att1 + a2) ---
            tb = sb1.tile([128, HCN * SLOTS * 128], BF16, tag="tanhbuf")
            tb_v = tb[:].rearrange("p (h s b) -> p h s b", h=HCN, s=SLOTS, b=128)
            sp0 = psum.tile([1, 512], F32, tag="sps0")
            sp1 = psum.tile([1, 384], F32, tag="sps1")
            for hc in range(HCN):
                a2p = psum.tile([128, 128], F32, tag="a2", bufs=2)
                for ec in range(HCN):
                    nc.tensor.matmul(a2p[:], ua_v[:, ec, hc], hT_prev[:, ec],
                                     start=(ec == 0), stop=(ec == HCN - 1))
                a2b = a2p[:].rearrange("p (o b) -> p o b", o=1, b=128).broadcast_to((128, SLOTS, 128))
                nc.vector.tensor_tensor(tb_v[:, hc], att1_v[:, hc], a2b, op=OP.add)
                nc.scalar.activation(tb_v[:, hc], tb_v[:, hc], AF.Tanh)
                tbf = tb_v[:, hc].rearrange("p s b -> p (s b)")
                nc.tensor.matmul(sp0[:], vat[:, hc:hc + 1], tbf[:, 0:512],
                                 start=(hc == 0), stop=(hc == HCN - 1))
                nc.tensor.matmul(sp1[:], vat[:, hc:hc + 1], tbf[:, 512:896],
                                 start=(hc == 0), stop=(hc == HCN - 1))
            # --- scores all-gather ---
            ssl = sb.tile([1, SLOTS * 128], F32, tag="ssl", bufs=1)
            nc.vector.tensor_copy(ssl[:, 0:512], sp0[:])
            nc.vector.tensor_copy(ssl[:, 512:896], sp1[:])
            sin = dram.tile([1, SLOTS * 128], F32, tag="sin")
            nc.sync.dma_start(sin[:], ssl[:])
            sout = dram.tile([RP, 128], F32, tag="sout")
            nc.gpsimd.collective_compute(
                "AllGather", OP.bypass, replica_groups=[list(range(NC))],
                ins=[sin[:].opt()], outs=[sout[:].opt()])
            ssb = sb.tile([RP, 128], F32, tag="ssb")
            nc.sync.dma_start(ssb[:], sout[:])
            if debug and t == 0:
                nc.sync.dma_start(dbg_tb[:], tb[:])
                nc.sync.dma_start(dbg_ssb[:], ssb[:])
            tps = psum.tile([128, RP], F32, tag="xpose")
            nc.tensor.transpose(tps[:], ssb[:], eye[0:RP, 0:RP])
            # --- softmax over the 49 real slots ---
            ex = sb.tile([128, R], F32, tag="ex")
            nc.scalar.activation(ex[:], tps[:, 0:R], AF.Exp)
            ssum = sb.tile([128, 1], F32, tag="ssum")
            nc.vector.tensor_reduce(ssum[:], ex[:], mybir.AxisListType.X, OP.add)
            rsum = sb.tile([128, 1], F32, tag="rsum")
            nc.vector.reciprocal(rsum[:], ssum[:])
            wgt = sb.tile([128, R], F32, tag="wgt")
            nc.vector.tensor_scalar_mul(wgt[:], ex[:], rsum[:])
            nc.sync.dma_start(w_out[t], wgt[:])

            # --- gates: bias + we@Wie + h@Whh + sum_r w_r * G_r ---
            wes = sb.tile([128, ECN * 128], BF16, tag="wes")
            nc.sync.dma_start(
                wes[:].rearrange("p (a b) -> p a b", a=ECN, b=128), weT[:, :, t, :])
            wes_v = wes[:].rearrange("p (a b) -> p a b", a=ECN, b=128)
            gp = psum.tile([128, JS], F32, tag="gp", bufs=2)
            nc.tensor.matmul(gp[:], ones_row[:], gbias_t[:], start=True, stop=False)
            for ec in range(ECN):
                nc.tensor.matmul(gp[:], wes_v[:, ec], wie_v[:, ec], start=False, stop=False)
            for ec in range(HCN):
                nc.tensor.matmul(gp[:], hT_prev[:, ec], whh_v[:, ec], start=False, stop=False)
            for s in range(R):
                dgt = dg_pool.tile([128, 128], BF16, tag="dg")
                if s % 3 == 2:
                    nc.scalar.mul(dgt[:], eyeb[:], wgt[:, s:s + 1])
                else:
                    nc.vector.tensor_scalar_mul(dgt[:], eyeb[:], wgt[:, s:s + 1])
                nc.tensor.matmul(gp[:], dgt[:], G_v[:, s], start=False, stop=(s == R - 1))

            if debug and t == 0:
                gdump = sb.tile([128, JS], F32, tag="gdump", bufs=1)
                nc.vector.tensor_copy(gdump[:], gp[:])
                nc.sync.dma_start(dbg_gates[:], gdump[:])
            # --- LSTM cell (local 64 channels), then BN1 ---
            ig = sb.tile([128, JS], F32, tag="ig")
            nc.scalar.activation(ig[:, 0:64], gp[:, 0:64], AF.Sigmoid)
            nc.scalar.activation(ig[:, 64:128], gp[:, 64:128], AF.Sigmoid)
            nc.scalar.activation(ig[:, 192:256], gp[:, 192:256], AF.Sigmoid)
            nc.scalar.activation(ig[:, 128:192], gp[:, 128:192], AF.Tanh)
            fc_ = sb.tile([128, HS], F32, tag="fc_")
            nc.vector.tensor_mul(fc_[:], ig[:, 64:128], c_prev)
            ig_ = sb.tile([128, HS], F32, tag="ig_")
            nc.vector.tensor_mul(ig_[:], ig[:, 0:64], ig[:, 128:192])
            c_new = sb.tile([128, HS], F32, tag="c")
            nc.vector.tensor_add(c_new[:], fc_[:], ig_[:])
            tc_ = sb.tile([128, HS], F32, tag="tc_")
            nc.scalar.activation(tc_[:], c_new[:], AF.Tanh)
            hraw = sb.tile([128, HS], BF16, tag="hraw")
            nc.vector.tensor_mul(hraw[:], ig[:, 192:256], tc_[:])
            # transpose to [64, 128]
            hps = psum.tile([128, 128], BF16, tag="xpose")
            nc.tensor.transpose(hps[0:HS, :], hraw[:], eyeb[:])
            hTr = hps[0:HS, :]
            # BN1 (stats over batch = free dim)
            bns = sb.tile([HS, 6], F32, tag="bns")
            nc.vector.bn_stats(bns[:], hTr)
            bna = sb.tile([HS, 2], F32, tag="bna")
            nc.vector.bn_aggr(bna[:], bns[:])
            sq = sb.tile([HS, 1], F32, tag="sq")
            nc.scalar.activation(sq[:], bna[:, 1:2], AF.Sqrt, bias=eps_t[0:HS, :])
            rstd = sb.tile([HS, 1], F32, tag="rstd")
            nc.vector.reciprocal(rstd[:], sq[:])
            seff = sb.tile([HS, 1], F32, tag="seff")
            nc.vector.tensor_mul(seff[:], rstd[:], g1_t[:])
            mse = sb.tile([HS, 1], F32, tag="mse")
            nc.vector.tensor_mul(mse[:], bna[:, 0:1], seff[:])
            beff = sb.tile([HS, 1], F32, tag="beff")
            nc.vector.tensor_sub(beff[:], be1_t[:], mse[:])
            hbn = sb.tile([HS, 128], BF16, tag="hbn")
            nc.scalar.activation(hbn[:], hTr, AF.Identity, bias=beff[:], scale=seff[:])
            # --- h all-gather ---
            hin = dram.tile([HS, 128], BF16, tag="hin")
            nc.sync.dma_start(hin[:], hbn[:])
            if debug and t == 0:
                nc.sync.dma_start(dbg_hbn[:], hbn[:])
            hout = dram.tile([H, 128], BF16, tag="hout")
            nc.gpsimd.collective_compute(
                "AllGather", OP.bypass, replica_groups=[list(range(NC))],
                ins=[hin[:].opt()], outs=[hout[:].opt()])
            hT_new = sb.tile([128, HCN * 128], BF16, tag="hT")
            nc.sync.dma_start(
                hT_new[:].rearrange("p (a b) -> p a b", a=HCN, b=128),
                hout[:].rearrange("(a p) b -> p a b", a=HCN, p=128))
            hT_new_v = hT_new[:].rearrange("p (a b) -> p a b", a=HCN, b=128)

            # --- fc -> relu -> BN2 -> fc2 (off critical path) ---
            o1 = sb.tile([128, 2 * 128], F32, tag="o1")
            for mc in range(2):
                fps = psum.tile([128, 512], F32, tag="fv")
                for ec in range(HCN):
                    nc.tensor.matmul(fps[:, 0:128], fcw_v[:, ec, mc], hT_new_v[:, ec],
                                     start=(ec == 0), stop=(ec == HCN - 1))
                nc.scalar.activation(o1[:, mc * 128:(mc + 1) * 128], fps[:, 0:128], AF.Relu,
                                     bias=fcb_t[:, mc:mc + 1])
            o1bn = sb.tile([128, 2 * 128], BF16, tag="o1bn")
            for mc in range(2):
                o1c = o1[:, mc * 128:(mc + 1) * 128]
                bns2 = sb.tile([128, 6], F32, tag="bns2")
                nc.vector.bn_stats(bns2[:], o1c)
                bna2 = sb.tile([128, 2], F32, tag="bna2")
                nc.vector.bn_aggr(bna2[:], bns2[:])
                sq2 = sb.tile([128, 1], F32, tag="sq2")
                nc.scalar.activation(sq2[:], bna2[:, 1:2], AF.Sqrt, bias=eps_t[:])
                rstd2 = sb.tile([128, 1], F32, tag="rstd2")
                nc.vector.reciprocal(rstd2[:], sq2[:])
                seff2 = sb.tile([128, 1], F32, tag="seff2")
                nc.vector.tensor_mul(seff2[:], rstd2[:], g2_t[:, mc:mc + 1])
                mse2 = sb.tile([128, 1], F32, tag="mse2")
                nc.vector.tensor_mul(mse2[:], bna2[:, 0:1], seff2[:])
                beff2 = sb.tile([128, 1], F32, tag="beff2")
                nc.vector.tensor_sub(beff2[:], be2_t[:, mc:mc + 1], mse2[:])
                nc.scalar.activation(o1bn[:, mc * 128:(mc + 1) * 128], o1c, AF.Identity,
                                     bias=beff2[:], scale=seff2[:])
            for n0, nn in ((0, 512), (512, 512), (1024, VS - 1024)):
                vps = psum.tile([128, nn], F32, tag="fv")
                for kc in range(2):
                    nc.tensor.matmul(vps[:], o1bn[:, kc * 128:(kc + 1) * 128],
                                     fc2_v[:, kc, n0:n0 + nn],
                                     start=(kc == 0), stop=(kc == 1))
                vsb = sb.tile([128, 512], F32, tag="vsb")
                nc.vector.tensor_tensor(vsb[:, 0:nn], vps[:], fc2b_t[:, n0:n0 + nn], op=OP.add)
                nc.sync.dma_start(out_s[t][:, n0:n0 + nn], vsb[:, 0:nn])

            hT_prev = hT_new_v
            c_prev = c_new[:]

    nc.compile()
    return nc


def _host_prep(captions, features, emb, Wa, ba, Ua, bu, va, bv,
               W_ih, b_ih, W_hh, b_hh, g1, be1, fc_w, fc_b,
               g2, be2, fc2_w, fc2_b, ih_w, ih_b, ic_w, ic_b):
    f = _f32
    features = f(features)
    emb = f(emb)
    cap = np.asarray(captions).astype(np.int64)

    mean_f = features.mean(axis=1)                      # [B, D]
    h0 = mean_f @ f(ih_w).T + f(ih_b)                   # [B, H]
    c0 = mean_f @ f(ic_w).T + f(ic_b)
    embed = emb[cap]                                    # [B, T, E]

    W_ih = f(W_ih)
    W_ie = W_ih[:, :E]
    W_id = W_ih[:, E:]
    W_hh = f(W_hh)
    gb = f(b_ih) + f(b_hh)

    # fT staged: [slot, dp, dc*128 + b]; slots 0..48 = global r, 49..55 per-core
    fT = features.transpose(2, 1, 0)                    # [D, R, B]
    fT_sl = fT.reshape(DCN, 128, R, B).transpose(2, 1, 0, 3)   # [R, dp, dc, B]
    fT_sl = fT_sl.reshape(R, 128, DCN * 128)

    WaT_full = f(Wa).T                                   # [D, H]
    UaT_full = f(Ua).T                                   # [H, H]
    fcwT_full = f(fc_w).T                                # [H, 256]
    babu_full = f(ba) + f(bu)                            # [H]
    va_full = f(va)[0]                                   # [H]

    in_maps = []
    for k in range(NC):
        rows = np.concatenate([q * H + np.arange(HS) + k * HS for q in range(4)])
        my_r = [7 * k + i for i in range(SLOTS)]
        fT_core = np.zeros((RP, 128, DCN * 128), np.float32)
        fT_core[:R] = fT_sl
        for i, r in enumerate(my_r):
            if r < R:
                fT_core[R + i] = fT_sl[r]

        m = dict(
            fTd=_bf(fT_core),
            WidT=_bf(W_id[rows].T.reshape(DCN, 128, JS).transpose(1, 0, 2)),
            WaT=_bf(WaT_full.reshape(DCN, 128, HCN, 128).transpose(1, 0, 2, 3)),
            weT=_bf(embed.transpose(2, 1, 0).reshape(ECN, 128, T, B).transpose(1, 0, 2, 3)),
            UaT=_bf(UaT_full.reshape(HCN, 128, HCN, 128).transpose(1, 0, 2, 3)),
            WieT=_bf(W_ie[rows].T.reshape(ECN, 128, JS).transpose(1, 0, 2)),
            WhhT=_bf(W_hh[rows].T.reshape(HCN, 128, JS).transpose(1, 0, 2)),
            gbias=_bf(gb[rows][None, :]),
            vaT=_bf(va_full.reshape(HCN, 128).T),
            babu=f(babu_full.reshape(HCN, 128).T),
            fcwT=_bf(fcwT_full.reshape(HCN, 128, 2, 128).transpose(1, 0, 2, 3)),
            fcb=f(f(fc_b).reshape(2, 128).T),
            g1s=f(f(g1)[k * HS:(k + 1) * HS][:, None]),
            be1s=f(f(be1)[k * HS:(k + 1) * HS][:, None]),
            g2t=f(f(g2).reshape(2, 128).T),
            be2t=f(f(be2).reshape(2, 128).T),
            fc2T=_bf(f(fc2_w)[k * VS:(k + 1) * VS].T.reshape(2, 128, VS).transpose(1, 0, 2)),
            fc2b=_bf(np.broadcast_to(f(fc2_b)[k * VS:(k + 1) * VS][None, :], (128, VS))),
            h0T=_bf(h0.T.reshape(HCN, 128, B).transpose(1, 0, 2)),
            c0s=f(c0[:, k * HS:(k + 1) * HS]),
            eye_in=np.eye(128, dtype=np.float32),
            eyeb_in=_bf(np.eye(128, dtype=np.float32)),
        )
        in_maps.append(m)
    return in_maps


def kernel(**inputs):
    if "nc" not in _cache:
        _cache["nc"] = build_nc()
    nc = _cache["nc"]
    in_maps = _host_prep(**inputs)
    trace = os.environ.get("KERNEL_TRACE", "0") == "1"
    res = run_bass_kernel_spmd(nc, in_maps, core_ids=list(range(NC)), trace=trace)
    _cache["last_exec_ns"] = res.exec_time_ns
    outs = [res.results[k]["out_s"] for k in range(NC)]       # each [t, 128, VS]
    outputs = np.concatenate(outs, axis=2).transpose(1, 0, 2)  # [B, t, V]
    atten = res.results[0]["w_out"].transpose(1, 0, 2)         # [B, t, R]
    if outputs.shape[1] < T:
        pass  # debug mode with fewer steps
    return outputs.astype(np.float32), atten.astype(np.float32)
